# revision 32
# baseline (speedup 1.0000x reference)
"""Trainium2 Bass kernel for nn_Attention_6073083756792.

The reference module is (faithfully) softmax-free: attn = sim = (q^T k), so the
whole attention block is linear in the normalized input.  Folding the RMSNorm
column scaling through the channel GEMMs collapses the entire module to

    y[:, j] = E_b @ xs[:, j] + b_out + x[:, j]      per batch b, where
    xs[:, j] = x[:, j] / ||x[:, j]||
    A_b  = sum_j xs_j xs_j^T                        (64 x 64 Gram matrix)
    E_b  = sum_h U_h @ A_b @ V_h                    (64 x 64)
    U_h  = W_out[:, h] @ WV_h          (host precomputed, weights only)
    V_h  = WK_h^T @ WQ_h               (host precomputed, weights only)

Device schedule per core (spatial columns sharded 8 ways, 512 cols/core/batch):
  phase 1: per-batch input DMAs on two queues (pipelined); PE transposes to
           j-major; ACT Square, DVE grouped reduce + reciprocal, ACT sqrt,
           one DVE mul -> xs (unit columns); Gram via stat = mov = xs;
           per-batch PSUM->SBUF copies on ACT/DVE.
  The Gram export and the final output use the SWDGE prepare/trigger path:
  descriptor generation runs on the gpsimd engine well ahead of time, so the
  triggered DMA costs only trigger + transfer + completion-sem instead of a
  full HWDGE issue cycle (~1.1us saved per hop).  cc_in / yout are pre-zeroed
  by early SWDGE copies so the scatter-ADD lands plain values.  (dma_gather
  is not functional under fake_nrt, so the import stays a plain DMA.)
  AllReduce (add) of the [128, 64] partial-Gram block (32 KB).
  phase 2: plain import DMA lands the reduced Gram in SBUF in (c, (b, k))
           layout via a strided AP; one stacked t-matmul; blkdiag(V_h,V_h)
           E-matmuls; y = [Ec ; I]^T @ [xs ; x] + b in float32r; output via a
           pre-prepared scatter triggered on the bias copies.
  The xs transpose-back path runs in slack time during the collective; junk
  matmuls keep PE out of the cold p-state across the collective window.
"""

import numpy as np

import concourse.bacc as bacc
import concourse.bass as bass
import concourse.mybir as mybir
import concourse.tile as tile
from concourse.bass_utils import run_bass_kernel_spmd
from concourse.masks import make_identity

F32 = mybir.dt.float32
F32R = mybir.dt.float32r
I16 = mybir.dt.int16
AF = mybir.ActivationFunctionType

N_CORES = 8
B = 2
C = 64          # channels (dim)
N = 4096        # spatial positions 16*16*16
NPC = N // N_CORES  # columns per core
NT = NPC // 128     # 128-column j-tiles per batch per core
HEADS = 4
DIM_HEAD = 32
HID = HEADS * DIM_HEAD
SCALE = DIM_HEAD ** -0.5
EPS = 1e-12     # torch F.normalize default (reference)

# packed const layout: [ucatT (256) | vflat (256) | bvec (1)]
WC_COLS = HEADS * C + HEADS * C + 1


def _emit(nc, tc, pools, tensors):
    data, small, consts, pst, psa, psb, psw, dram = pools
    xin, yout, wconst, auxidx = (
        tensors["xin"], tensors["yout"], tensors["wconst"], tensors["auxidx"])
    collective = tensors["collective"]
    warm_big, warm_pre = tensors["warm_big"], tensors["warm_pre"]

    yout_rows = yout[:, :, :].rearrange("b c n -> (b c) n")

    # ---- constants / setup (emitted first; engines idle before input lands)
    ident = consts.tile([128, 128], F32)
    make_identity(nc, ident[:, :])                      # Pool memset+select
    identr = consts.tile([128, 128], F32R)
    nc.vector.tensor_copy(identr[:, :], ident[:, :])    # DVE

    idx_sb = consts.tile([128, 16], I16)

    wc_sb = consts.tile([C, WC_COLS], F32R)
    b_sb = wc_sb[:, 2 * HEADS * C:2 * HEADS * C + 1].bitcast(F32)

    # [Ec ; I] stationaries: identity halves via partition-shifted DVE copies
    lzs = []
    for b in range(B):
        lz = consts.tile([2 * C, C], F32R, tag=f"lz{b}")
        nc.vector.tensor_copy(lz[C:2 * C, :], ident[0:C, 0:C])
        lzs.append(lz)

    zeros = consts.tile([128, NPC], F32)
    nc.vector.memset(zeros[:, :], 0.0)

    # preload the sqrt_and_others ACT table (covers Sqrt, Square, Identity,
    # Copy) while input DMAs are in flight
    warm = consts.tile([1, 1], F32)
    nc.vector.memset(warm[:, :], 0.0)
    nc.scalar.sqrt(warm[:, :], warm[:, :])

    # ---- input DMAs: one per batch on two queues so batch-0 compute starts
    # while batch-1 is still in flight
    z_sb = data.tile([2 * C, B * NPC], F32R, tag="z")
    nc.sync.dma_start(z_sb[C:2 * C, 0:NPC], xin[0, :, :].bitcast(F32R))
    nc.sync.dma_start(z_sb[C:2 * C, NPC:2 * NPC], xin[1, :, :].bitcast(F32R))
    nc.sync.dma_start(idx_sb[:, :], auxidx[:, :])

    # DRAM round-trip buffers for the collective ([128 rows, 64] layout:
    # row 2c+b holds A_b[c, :])
    cc_in = dram.tile([2 * C, C], F32, tag="cc_in")
    cc_out = dram.tile([2 * C, C], F32, tag="cc_out")

    # pre-zero cc_in / yout (plain SWDGE copies; the exports below ADD)
    nc.gpsimd.dma_start(cc_in[:, :], zeros[:, 0:C])
    nc.gpsimd.dma_start(yout_rows, zeros[:, :])
    nc.sync.dma_start(wc_sb[:, :], wconst[:, :].bitcast(F32R))

    # early PE warm-up: ramp the tensor engine before the input lands
    warm_ps = psw.tile([C, C], F32, tag="junk")
    for _ in range(warm_pre):
        nc.tensor.matmul(warm_ps[:, :], ident[0:C, 0:C], ident[0:C, 0:C],
                         start=True, stop=True)

    # ---- phase 1 ----
    xT_pss, sq_sbs, ss_sbs, inv_sbs, xs_sbs = [], [], [], [], []
    for b in range(B):
        xT_ps = pst.tile([128, NT * C], F32R, tag="xT")
        for i in range(NT):
            nc.tensor.transpose(
                xT_ps[:, i * C:(i + 1) * C],
                z_sb[C:2 * C, b * NPC + i * 128:b * NPC + (i + 1) * 128],
                identr[C:128, C:128],
            )
        xT_pss.append(xT_ps)

    for b in range(B):
        sq = data.tile([128, NT * C], F32, tag="sq")
        nc.scalar.activation(sq[:, :], xT_pss[b][:, :].bitcast(F32), AF.Square)
        sq_sbs.append(sq)

    for b in range(B):
        ss = small.tile([128, NT], F32, tag="ss")
        nc.vector.tensor_reduce(
            ss[:, :],
            sq_sbs[b][:, :].rearrange("p (g k) -> p g k", g=NT),
            mybir.AxisListType.X,
            mybir.AluOpType.add,
        )
        ss_sbs.append(ss)

    for b in range(B):
        sroot = small.tile([128, NT], F32, tag="sroot")
        nc.scalar.sqrt(sroot[:, :], ss_sbs[b][:, :])    # ||x_j|| on ACT
        inv_sbs.append(sroot)

    invr_sbs = []
    for b in range(B):
        invr = small.tile([128, NT], F32, tag="invr")
        nc.vector.reciprocal(invr[:, :], inv_sbs[b][:, :])  # 1/||x_j|| on DVE
        invr_sbs.append(invr)

    for b in range(B):
        xs = data.tile([128, NT * C], F32, tag="xs")
        nc.vector.tensor_mul(
            xs[:, :].rearrange("p (g k) -> p g k", g=NT),
            xT_pss[b][:, :].bitcast(F32).rearrange("p (g k) -> p g k", g=NT),
            invr_sbs[b][:, :].unsqueeze(2).broadcast_to((128, NT, C)),
        )
        xs_sbs.append(xs)

    a_pss = []
    for b in range(B):
        a_ps = psa.tile([C, C], F32, tag=f"A{b}")
        for i in range(NT):
            nc.tensor.matmul(
                a_ps[:, :],
                xs_sbs[b][:, i * C:(i + 1) * C],
                xs_sbs[b][:, i * C:(i + 1) * C],
                start=(i == 0), stop=(i == NT - 1),
            )
        a_pss.append(a_ps)

    # partial-Gram staging: batch halves on parallel engines (partition-
    # stacked [2C, C] so one export covers both)
    cc_sb = small.tile([2 * C, C], F32, tag="cc_sb")
    nc.scalar.copy(cc_sb[0:C, :], a_pss[0][:, :])
    nc.vector.tensor_copy(cc_sb[C:2 * C, :], a_pss[1][:, :])

    # export: prepared scatter-add (desc-gen ran long ago on gpsimd), fired
    # the moment both Gram halves are staged.  src partition (b*64+c) lands
    # on cc_in row (2c+b).
    sem_exp = nc.alloc_semaphore("exp_dma")
    nc.gpsimd.dma_scatter_add(
        cc_in[:, :], cc_sb[:, :].rearrange("p (n e) -> p n e", n=1),
        idx_sb[:, 8:16], 128, 128, C,
        prepare_only=True, sem=sem_exp)
    nc.gpsimd.trigger_dma(count=None)

    if collective:
        nc.gpsimd.collective_compute(
            "AllReduce",
            mybir.AluOpType.add,
            replica_groups=[list(range(N_CORES))],
            ins=[cc_in.opt()],
            outs=[cc_out.opt()],
        )
    else:
        # timing-model variant: stand-in DMA instead of the collective
        nc.sync.dma_start(cc_out[:, :], cc_in[:, :])

    # import: plain DMA; row (2c+b) lands at a_il[c, b, :], giving the
    # batch-stacked stationary [c, (b k)] for the t-matmul in SBUF.
    a_il = small.tile([C, B * C], F32R, tag="a_il")
    nc.sync.dma_start(
        a_il[:, :].rearrange("p (n e) -> p n e", n=B),
        cc_out[:, :].bitcast(F32R).rearrange("(c two) k -> c two k", two=2),
    )

    # vblk / bias setup on the gpsimd engine in its slack window (needs wconst)
    vblk = consts.tile([2 * C, HEADS * 2 * C], F32)
    wcv = wc_sb[:, HEADS * C:2 * HEADS * C].bitcast(F32).rearrange(
        "p (h k) -> p h k", h=HEADS)
    nc.vector.memset(vblk[:, :], 0.0)
    nc.scalar.copy(
        vblk[0:C, :].rearrange("p (h two k) -> p h two k", h=HEADS, two=2)[:, :, 0, :],
        wcv)
    nc.scalar.copy(
        vblk[C:2 * C, :].rearrange("p (h two k) -> p h two k", h=HEADS, two=2)[:, :, 1, :],
        wcv)
    b_sb2 = consts.tile([2 * C, 1], F32)
    nc.gpsimd.tensor_copy(b_sb2[0:C, :], b_sb)
    nc.gpsimd.tensor_copy(b_sb2[C:2 * C, :], b_sb)

    # slack path: xs transposed back to channel-major into z rows 0:64 (only
    # needed by the post-collective apply)
    for b in range(B):
        tb_ps = psb.tile([C, NPC], F32, tag="tb")
        for i in range(NT):
            nc.tensor.transpose(
                tb_ps[:, i * 128:(i + 1) * 128],
                xs_sbs[b][:, i * C:(i + 1) * C],
                ident[:, :],
            )
        if b == 0:
            nc.scalar.copy(z_sb[0:C, 0:NPC], tb_ps[:, :])
        else:
            nc.vector.tensor_copy(z_sb[0:C, NPC:2 * NPC], tb_ps[:, :])

    # PE keep-warm filler across the collective window (the cost model drops
    # PE to a cold p-state after ~3us idle).  Gated on cc_sb so they cannot
    # preempt phase-1 PE work; WAW-serialized on one scratch ring.
    for _ in range(warm_big):
        junk = psw.tile([C, C], F32, tag="junk")
        nc.tensor.matmul(junk[:, :], cc_sb[0:C, 0:C], cc_sb[0:C, 0:C],
                         start=True, stop=True)

    # ---- phase 2: E chain + apply ----
    t_full = pst.tile([128, HEADS * C], F32R, tag="xT")
    t_ps = t_full[:, :].bitcast(F32)
    nc.tensor.matmul(t_ps, a_il[:, :], wc_sb[:, 0:HEADS * C])
    t_sb = small.tile([128, HEADS * C], F32, tag="t_sb")
    nc.scalar.copy(t_sb[:, :], t_ps[:, :])

    e_both = psa.tile([2 * C, C], F32, tag="A0")
    for h in range(HEADS):
        nc.tensor.matmul(
            e_both[:, :],
            vblk[:, h * 2 * C:(h + 1) * 2 * C],
            t_sb[:, h * C:(h + 1) * C],
            start=(h == 0), stop=(h == HEADS - 1),
        )
    nc.scalar.copy(lzs[0][0:C, :], e_both[0:C, :])
    nc.vector.tensor_copy(lzs[1][0:C, :], e_both[C:2 * C, :])

    y_pss = []
    for b in range(B):
        y_ps = psb.tile([C, NPC], F32, tag="tb")
        nc.tensor.matmul(y_ps[:, :], lzs[b][:, :],
                         z_sb[:, b * NPC:(b + 1) * NPC])
        y_pss.append(y_ps)

    # bias + PSUM->SBUF staging on parallel engines into one [128, 512] tile
    ybig = data.tile([2 * C, NPC], F32, tag="ybig")
    nc.scalar.activation(ybig[0:C, :], y_pss[0][:, :], AF.Identity,
                         bias=b_sb2[0:C, :], scale=1.0)
    nc.vector.tensor_scalar_add(ybig[C:2 * C, :], y_pss[1][:, :],
                                b_sb2[0:C, :])

    # output: prepared scatter-add into the pre-zeroed yout, triggered on the
    # bias copies
    sem_y = nc.alloc_semaphore("y_dma")
    nc.gpsimd.dma_scatter_add(
        yout_rows, ybig[:, :].rearrange("p (n e) -> p n e", n=1),
        idx_sb[:, 0:8], 128, 128, NPC,
        prepare_only=True, sem=sem_y)
    nc.gpsimd.trigger_dma(count=None)

    if tensors.get("dbg"):
        dbgs = tensors["dbg"]
        nc.sync.dma_start(dbgs["d_ccsb"][:, :], cc_sb[:, :])
        nc.sync.dma_start(dbgs["d_ccout"][:, :], cc_out[:, :])
        nc.sync.dma_start(dbgs["d_ail"][:, :], a_il[:, :].bitcast(F32))
        nc.sync.dma_start(dbgs["d_tsb"][:, :], t_sb[:, :])
        nc.sync.dma_start(dbgs["d_lz0"][:, :], lzs[0][:, :].bitcast(F32))
        nc.sync.dma_start(dbgs["d_lz1"][:, :], lzs[1][:, :].bitcast(F32))
        nc.sync.dma_start(dbgs["d_z"][:, :], z_sb[:, :].bitcast(F32))
        nc.sync.dma_start(dbgs["d_ybig"][:, :], ybig[:, :])
        nc.sync.dma_start(dbgs["d_xs0"][:, :], xs_sbs[0][:, :])
        nc.sync.dma_start(dbgs["d_inv0"][:, :], invr_sbs[0][:, :])


def build_kernel(loops=1, collective=True, warm_big=24, warm_pre=2, dbg=False):
    nc = bacc.Bacc("TRN2", target_bir_lowering=False, debug=False,
                   num_devices=N_CORES)

    xin = nc.dram_tensor("xin", [B, C, NPC], F32, kind="ExternalInput")
    wconst = nc.dram_tensor("wconst", [C, WC_COLS], F32, kind="ExternalInput")
    auxidx = nc.dram_tensor("auxidx", [128, 16], I16, kind="ExternalInput")
    yout = nc.dram_tensor("yout", [B, C, NPC], F32, kind="ExternalOutput")
    dbgs = None
    if dbg:
        dbgs = {
            "d_ccsb": nc.dram_tensor("d_ccsb", [128, 64], F32, kind="ExternalOutput"),
            "d_ccout": nc.dram_tensor("d_ccout", [128, 64], F32, kind="ExternalOutput"),
            "d_ail": nc.dram_tensor("d_ail", [64, 128], F32, kind="ExternalOutput"),
            "d_tsb": nc.dram_tensor("d_tsb", [128, 256], F32, kind="ExternalOutput"),
            "d_lz0": nc.dram_tensor("d_lz0", [128, 64], F32, kind="ExternalOutput"),
            "d_lz1": nc.dram_tensor("d_lz1", [128, 64], F32, kind="ExternalOutput"),
            "d_z": nc.dram_tensor("d_z", [128, 1024], F32, kind="ExternalOutput"),
            "d_ybig": nc.dram_tensor("d_ybig", [128, 512], F32, kind="ExternalOutput"),
            "d_xs0": nc.dram_tensor("d_xs0", [128, 256], F32, kind="ExternalOutput"),
            "d_inv0": nc.dram_tensor("d_inv0", [128, 4], F32, kind="ExternalOutput"),
        }

    with tile.TileContext(nc) as tc:
        with (
            tc.tile_pool(name="consts", bufs=1) as consts,
            tc.tile_pool(name="data", bufs=2) as data,
            tc.tile_pool(name="small", bufs=2) as small,
            tc.tile_pool(name="pst", bufs=2, space="PSUM") as pst,
            tc.tile_pool(name="psa", bufs=1, space="PSUM") as psa,
            tc.tile_pool(name="psb", bufs=2, space="PSUM") as psb,
            tc.tile_pool(name="psw", bufs=2, space="PSUM") as psw,
            tc.tile_pool(name="dram", bufs=1, space="DRAM") as dram,
        ):
            pools = (data, small, consts, pst, psa, psb, psw, dram)
            tensors = {
                "xin": xin, "yout": yout, "wconst": wconst, "auxidx": auxidx,
                "collective": collective, "dbg": dbgs,
                "warm_big": warm_big, "warm_pre": warm_pre,
            }
            for _ in range(loops):
                _emit(nc, tc, pools, tensors)

    nc.compile()
    return nc


_NC_CACHE = {}


def _get_nc(loops=1, collective=True):
    key = (loops, collective)
    if key not in _NC_CACHE:
        _NC_CACHE[key] = build_kernel(loops=loops, collective=collective)
    return _NC_CACHE[key]


def _host_weights(g, w_qkv, w_out, b_out):
    Wp = w_qkv.astype(np.float64) * (8.0 * g.astype(np.float64))[None, :]
    WQ = Wp[0:HID] * SCALE
    WK = Wp[HID:2 * HID]
    WV = Wp[2 * HID:3 * HID]
    U = np.stack([
        w_out[:, 32 * h:32 * h + 32].astype(np.float64) @ WV[32 * h:32 * h + 32]
        for h in range(HEADS)
    ])  # [4, 64, 64], U_h = W_out_h @ WV_h
    V = np.stack([
        WK[32 * h:32 * h + 32].T @ WQ[32 * h:32 * h + 32]
        for h in range(HEADS)
    ])  # [4, 64, 64]
    wc = np.zeros((C, WC_COLS), dtype=np.float32)
    for h in range(HEADS):
        wc[:, h * C:(h + 1) * C] = U[h].T.astype(np.float32)          # ucatT
        wc[:, HEADS * C + h * C:HEADS * C + (h + 1) * C] = V[h].astype(np.float32)
    wc[:, 2 * HEADS * C] = np.asarray(b_out, np.float32)
    return np.ascontiguousarray(wc)


def _aux_idx():
    """SWDGE index tables, [128, 16] int16.

    cols 0:8  — identity map: listed index i -> row i        (gather/yout)
    cols 8:16 — export map: src partition i=(b*64+c) -> row (2c+b)
    The engine reads idx(i) from [i%16, i//16]; partitions 16.. are unread
    but must hold in-range values.
    """
    idx = np.zeros((128, 16), dtype=np.int16)
    for p in range(128):
        for s in range(8):
            i = s * 16 + (p % 16)
            idx[p, s] = i
            idx[p, 8 + s] = 2 * (i % 64) + (i // 64)
    return idx


def _in_maps(x, g, w_qkv, w_out, b_out):
    x = np.asarray(x, dtype=np.float32)
    b, c, h, w, d = x.shape
    n = h * w * d
    xf = np.ascontiguousarray(x.reshape(b, c, n))
    wc = _host_weights(
        np.asarray(g, np.float32), np.asarray(w_qkv, np.float32),
        np.asarray(w_out, np.float32), np.asarray(b_out, np.float32))
    aux = _aux_idx()
    maps = []
    for core in range(N_CORES):
        sl = np.ascontiguousarray(xf[:, :, core * NPC:(core + 1) * NPC])
        maps.append({"xin": sl, "wconst": wc, "auxidx": aux})
    return maps, (b, c, h, w, d, n)


def _gather_out(res, shape):
    b, c, h, w, d, n = shape
    out = np.empty((b, c, n), dtype=np.float32)
    for core in range(N_CORES):
        out[:, :, core * NPC:(core + 1) * NPC] = res.results[core]["yout"]
    return out.reshape(b, c, h, w, d)


def kernel(x, g, w_qkv, w_out, b_out, **_unused):
    maps, shape = _in_maps(x, g, w_qkv, w_out, b_out)
    nc = _get_nc()
    res = run_bass_kernel_spmd(nc, maps, core_ids=list(range(N_CORES)))
    return _gather_out(res, shape)


def run_variant(x, g, w_qkv, w_out, b_out, loops=1, collective=True, **kwargs):
    """Run a loop/collective variant; returns (out, BassKernelResults)."""
    maps, shape = _in_maps(x, g, w_qkv, w_out, b_out)
    nc = _get_nc(loops=loops, collective=collective)
    res = run_bass_kernel_spmd(nc, maps, core_ids=list(range(N_CORES)), **kwargs)
    return _gather_out(res, shape), res


# revision 36
# speedup vs baseline: 1.0129x; 1.0129x over previous
"""Trainium2 Bass kernel for nn_Attention_6073083756792.

The reference module is (faithfully) softmax-free: attn = sim = (q^T k), so the
whole attention block is linear in the normalized input.  Folding the RMSNorm
column scaling through the channel GEMMs collapses the entire module to

    y[:, j] = E_b @ xs[:, j] + b_out + x[:, j]      per batch b, where
    xs[:, j] = x[:, j] / ||x[:, j]||
    A_b  = sum_j xs_j xs_j^T                        (64 x 64 Gram matrix)
    E_b  = sum_h U_h @ A_b @ V_h                    (64 x 64)
    U_h  = W_out[:, h] @ WV_h          (host precomputed, weights only)
    V_h  = WK_h^T @ WQ_h               (host precomputed, weights only)

Device schedule per core (spatial columns sharded 8 ways, 512 cols/core/batch):
  phase 1: per-batch input DMAs on two queues (pipelined); PE transposes to
           j-major; ACT Square, DVE grouped reduce + reciprocal, ACT sqrt,
           one DVE mul -> xs (unit columns); Gram via stat = mov = xs;
           per-batch PSUM->SBUF copies on ACT/DVE.
  The Gram export and the final output use the SWDGE prepare/trigger path:
  descriptor generation runs on the gpsimd engine well ahead of time, so the
  triggered DMA costs only trigger + transfer + completion-sem instead of a
  full HWDGE issue cycle (~1.1us saved per hop).  cc_in / yout are pre-zeroed
  by early SWDGE copies so the scatter-ADD lands plain values.  (dma_gather
  is not functional under fake_nrt, so the import stays a plain DMA.)
  AllReduce (add) of the [128, 64] partial-Gram block (32 KB).
  phase 2: plain import DMA lands the reduced Gram in SBUF in (c, (b, k))
           layout via a strided AP; one stacked t-matmul; blkdiag(V_h,V_h)
           E-matmuls; y = [Ec ; I]^T @ [xs ; x] + b in float32r; output via a
           pre-prepared scatter triggered on the bias copies.
  The xs transpose-back path runs in slack time during the collective; junk
  matmuls keep PE out of the cold p-state across the collective window.
"""

import numpy as np

import concourse.bacc as bacc
import concourse.bass as bass
import concourse.mybir as mybir
import concourse.tile as tile
from concourse.bass_utils import run_bass_kernel_spmd
from concourse.masks import make_identity

F32 = mybir.dt.float32
F32R = mybir.dt.float32r
BF16 = mybir.dt.bfloat16
I16 = mybir.dt.int16
AF = mybir.ActivationFunctionType

N_CORES = 8
B = 2
C = 64          # channels (dim)
N = 4096        # spatial positions 16*16*16
NPC = N // N_CORES  # columns per core
NT = NPC // 128     # 128-column j-tiles per batch per core
HEADS = 4
DIM_HEAD = 32
HID = HEADS * DIM_HEAD
SCALE = DIM_HEAD ** -0.5
EPS = 1e-12     # torch F.normalize default (reference)

# packed const layout: [ucatT (256) | vflat (256) | bvec (1)]
WC_COLS = HEADS * C + HEADS * C + 1


def _emit(nc, tc, pools, tensors):
    data, small, consts, pst, psa, psb, psw, dram = pools
    xin, yout, wconst, auxidx = (
        tensors["xin"], tensors["yout"], tensors["wconst"], tensors["auxidx"])
    collective = tensors["collective"]
    warm_big, warm_pre = tensors["warm_big"], tensors["warm_pre"]

    yout_rows = yout[:, :, :].rearrange("b c n -> (b c) n")

    # ---- constants / setup (emitted first; engines idle before input lands)
    ident = consts.tile([128, 128], F32)
    make_identity(nc, ident[:, :])                      # Pool memset+select
    identr = consts.tile([128, 128], F32R)
    nc.vector.tensor_copy(identr[:, :], ident[:, :])    # DVE

    idx_sb = consts.tile([128, 32], I16)

    wc_sb = consts.tile([C, WC_COLS], F32R)
    b_sb = wc_sb[:, 2 * HEADS * C:2 * HEADS * C + 1].bitcast(F32)

    # [Ec ; I] stationaries: identity halves via partition-shifted DVE copies
    lzs = []
    for b in range(B):
        lz = consts.tile([2 * C, C], F32R, tag=f"lz{b}")
        nc.vector.tensor_copy(lz[C:2 * C, :], ident[0:C, 0:C])
        lzs.append(lz)

    zeros = consts.tile([128, NPC], F32)
    nc.vector.memset(zeros[:, :], 0.0)

    # preload the sqrt_and_others ACT table (covers Sqrt, Square, Identity,
    # Copy) while input DMAs are in flight
    warm = consts.tile([1, 1], F32)
    nc.vector.memset(warm[:, :], 0.0)
    nc.scalar.sqrt(warm[:, :], warm[:, :])

    # ---- input DMAs: one per batch on two queues so batch-0 compute starts
    # while batch-1 is still in flight
    z_sb = data.tile([2 * C, B * NPC], F32R, tag="z")
    nc.sync.dma_start(z_sb[C:2 * C, 0:NPC], xin[0, :, :].bitcast(F32R))
    nc.sync.dma_start(z_sb[C:2 * C, NPC:2 * NPC], xin[1, :, :].bitcast(F32R))
    nc.sync.dma_start(idx_sb[:, :], auxidx[:, :])

    # DRAM round-trip buffers for the collective ([128 rows, 64] layout:
    # row 2c+b holds A_b[c, :])
    cc_in = dram.tile([2 * C, C], F32, tag="cc_in")
    cc_out = dram.tile([2 * C, C], F32, tag="cc_out")

    # pre-zero cc_in / yout (plain SWDGE copies; the exports below ADD)
    nc.gpsimd.dma_start(cc_in[:, :], zeros[:, 0:C])
    nc.gpsimd.dma_start(yout_rows, zeros[:, :])
    nc.sync.dma_start(wc_sb[:, :], wconst[:, :].bitcast(F32R))

    # early PE warm-up: ramp the tensor engine before the input lands
    warm_ps = psw.tile([C, C], F32, tag="junk")
    for _ in range(warm_pre):
        nc.tensor.matmul(warm_ps[:, :], ident[0:C, 0:C], ident[0:C, 0:C],
                         start=True, stop=True)

    # ---- phase 1 ----
    xT_pss, sq_sbs, ss_sbs, inv_sbs, xs_sbs = [], [], [], [], []
    for b in range(B):
        xT_ps = pst.tile([128, NT * C], F32R, tag="xT")
        for i in range(NT):
            nc.tensor.transpose(
                xT_ps[:, i * C:(i + 1) * C],
                z_sb[C:2 * C, b * NPC + i * 128:b * NPC + (i + 1) * 128],
                identr[C:128, C:128],
            )
        xT_pss.append(xT_ps)

    for b in range(B):
        sq = data.tile([128, NT * C], F32, tag="sq")
        nc.scalar.activation(sq[:, :], xT_pss[b][:, :].bitcast(F32), AF.Square)
        sq_sbs.append(sq)

    for b in range(B):
        ss = small.tile([128, NT], F32, tag="ss")
        nc.vector.tensor_reduce(
            ss[:, :],
            sq_sbs[b][:, :].rearrange("p (g k) -> p g k", g=NT),
            mybir.AxisListType.X,
            mybir.AluOpType.add,
        )
        ss_sbs.append(ss)

    for b in range(B):
        sroot = small.tile([128, NT], F32, tag="sroot")
        nc.scalar.sqrt(sroot[:, :], ss_sbs[b][:, :])    # ||x_j|| on ACT
        inv_sbs.append(sroot)

    invr_sbs = []
    for b in range(B):
        invr = small.tile([128, NT], F32, tag="invr")
        nc.vector.reciprocal(invr[:, :], inv_sbs[b][:, :])  # 1/||x_j|| on DVE
        invr_sbs.append(invr)

    for b in range(B):
        xs = data.tile([128, NT * C], F32, tag="xs")
        nc.vector.tensor_mul(
            xs[:, :].rearrange("p (g k) -> p g k", g=NT),
            xT_pss[b][:, :].bitcast(F32).rearrange("p (g k) -> p g k", g=NT),
            invr_sbs[b][:, :].unsqueeze(2).broadcast_to((128, NT, C)),
        )
        xs_sbs.append(xs)

    a_pss = []
    for b in range(B):
        a_ps = psa.tile([C, C], F32, tag=f"A{b}")
        for i in range(NT):
            nc.tensor.matmul(
                a_ps[:, :],
                xs_sbs[b][:, i * C:(i + 1) * C],
                xs_sbs[b][:, i * C:(i + 1) * C],
                start=(i == 0), stop=(i == NT - 1),
            )
        a_pss.append(a_ps)

    # partial-Gram staging: batch halves on parallel engines (partition-
    # stacked [2C, C] so one export covers both)
    cc_sb = small.tile([2 * C, C], F32, tag="cc_sb")
    nc.scalar.copy(cc_sb[0:C, :], a_pss[0][:, :])
    nc.vector.tensor_copy(cc_sb[C:2 * C, :], a_pss[1][:, :])

    # export: prepared scatter-add (desc-gen ran long ago on gpsimd), fired
    # the moment both Gram halves are staged.  src partition (b*64+c) lands
    # on cc_in row (2c+b).
    sem_exp = nc.alloc_semaphore("exp_dma")
    nc.gpsimd.dma_scatter_add(
        cc_in[:, :], cc_sb[:, :].rearrange("p (n e) -> p n e", n=1),
        idx_sb[:, 8:16], 128, 128, C,
        prepare_only=True, sem=sem_exp)
    nc.gpsimd.trigger_dma(count=None)

    if collective:
        nc.gpsimd.collective_compute(
            "AllReduce",
            mybir.AluOpType.add,
            replica_groups=[list(range(N_CORES))],
            ins=[cc_in.opt()],
            outs=[cc_out.opt()],
        )
    else:
        # timing-model variant: stand-in DMA instead of the collective
        nc.sync.dma_start(cc_out[:, :], cc_in[:, :])

    # import: plain DMA; row (2c+b) lands at a_il[c, b, :], giving the
    # batch-stacked stationary [c, (b k)] for the t-matmul in SBUF.
    a_il = small.tile([C, B * C], F32R, tag="a_il")
    nc.sync.dma_start(
        a_il[:, :].rearrange("p (n e) -> p n e", n=B),
        cc_out[:, :].bitcast(F32R).rearrange("(c two) k -> c two k", two=2),
    )

    # vblk / bias setup on the gpsimd engine in its slack window (needs wconst)
    vblk = consts.tile([2 * C, HEADS * 2 * C], BF16)
    wcv = wc_sb[:, HEADS * C:2 * HEADS * C].bitcast(F32).rearrange(
        "p (h k) -> p h k", h=HEADS)
    nc.vector.memset(vblk[:, :], 0.0)
    nc.scalar.copy(
        vblk[0:C, :].rearrange("p (h two k) -> p h two k", h=HEADS, two=2)[:, :, 0, :],
        wcv)
    nc.scalar.copy(
        vblk[C:2 * C, :].rearrange("p (h two k) -> p h two k", h=HEADS, two=2)[:, :, 1, :],
        wcv)
    b_sb2 = consts.tile([2 * C, 1], F32)
    nc.gpsimd.tensor_copy(b_sb2[0:C, :], b_sb)
    nc.gpsimd.tensor_copy(b_sb2[C:2 * C, :], b_sb)

    # slack path: xs transposed back to channel-major into z rows 0:64 (only
    # needed by the post-collective apply)
    for b in range(B):
        tb_ps = psb.tile([C, NPC], F32, tag="tb")
        for i in range(NT):
            nc.tensor.transpose(
                tb_ps[:, i * 128:(i + 1) * 128],
                xs_sbs[b][:, i * C:(i + 1) * C],
                ident[:, :],
            )
        if b == 0:
            nc.scalar.copy(z_sb[0:C, 0:NPC], tb_ps[:, :])
        else:
            nc.vector.tensor_copy(z_sb[0:C, NPC:2 * NPC], tb_ps[:, :])

    # PE keep-warm filler across the collective window (the cost model drops
    # PE to a cold p-state after ~3us idle).  Gated on cc_sb so they cannot
    # preempt phase-1 PE work; WAW-serialized on one scratch ring.
    for _ in range(warm_big):
        junk = psw.tile([C, C], F32, tag="junk")
        nc.tensor.matmul(junk[:, :], cc_sb[0:C, 0:C], cc_sb[0:C, 0:C],
                         start=True, stop=True)

    # ---- phase 2: E chain + apply ----
    t_full = pst.tile([128, HEADS * C], F32R, tag="xT")
    t_ps = t_full[:, :].bitcast(F32)
    nc.tensor.matmul(t_ps, a_il[:, :], wc_sb[:, 0:HEADS * C])
    t_sb = small.tile([128, HEADS * C], BF16, tag="t_sb")
    nc.scalar.copy(t_sb[:, :], t_ps[:, :])

    e_both = psa.tile([2 * C, C], F32, tag="A0")
    for h in range(HEADS):
        nc.tensor.matmul(
            e_both[:, :],
            vblk[:, h * 2 * C:(h + 1) * 2 * C],
            t_sb[:, h * C:(h + 1) * C],
            start=(h == 0), stop=(h == HEADS - 1),
        )
    nc.scalar.copy(lzs[1][0:C, :], e_both[C:2 * C, :])
    nc.vector.tensor_copy(lzs[0][0:C, :], e_both[0:C, :])

    y_pss = [None, None]
    for b in (1, 0):
        y_ps = psb.tile([C, NPC], F32, tag="tb")
        nc.tensor.matmul(y_ps[:, :], lzs[b][:, :],
                         z_sb[:, b * NPC:(b + 1) * NPC])
        y_pss[b] = y_ps

    # bias + PSUM->SBUF staging on parallel engines into one [128, 512] tile
    ybig = data.tile([2 * C, NPC], F32, tag="ybig")
    nc.scalar.activation(ybig[0:C, :], y_pss[0][:, :], AF.Identity,
                         bias=b_sb2[0:C, :], scale=1.0)
    nc.vector.tensor_scalar_add(ybig[C:2 * C, :], y_pss[1][:, :],
                                b_sb2[0:C, :])

    # output: prepared scatter-add into the pre-zeroed yout, triggered on the
    # bias copies.  (Per-batch scatters were tried and are slower: Tile WAW-
    # serializes same-tensor scatters through the full DMA completion.)
    sem_y = nc.alloc_semaphore("y_dma")
    nc.gpsimd.dma_scatter_add(
        yout_rows, ybig[:, :].rearrange("p (n e) -> p n e", n=1),
        idx_sb[:, 0:8], 128, 128, NPC,
        prepare_only=True, sem=sem_y)
    nc.gpsimd.trigger_dma(count=None)

    if tensors.get("dbg"):
        dbgs = tensors["dbg"]
        nc.sync.dma_start(dbgs["d_ccsb"][:, :], cc_sb[:, :])
        nc.sync.dma_start(dbgs["d_ccout"][:, :], cc_out[:, :])
        nc.sync.dma_start(dbgs["d_ail"][:, :], a_il[:, :].bitcast(F32))
        nc.sync.dma_start(dbgs["d_tsb"][:, :], t_sb[:, :])
        nc.sync.dma_start(dbgs["d_lz0"][:, :], lzs[0][:, :].bitcast(F32))
        nc.sync.dma_start(dbgs["d_lz1"][:, :], lzs[1][:, :].bitcast(F32))
        nc.sync.dma_start(dbgs["d_z"][:, :], z_sb[:, :].bitcast(F32))
        nc.sync.dma_start(dbgs["d_ybig"][:, :], ybig[:, :])
        nc.sync.dma_start(dbgs["d_xs0"][:, :], xs_sbs[0][:, :])
        nc.sync.dma_start(dbgs["d_inv0"][:, :], invr_sbs[0][:, :])


def build_kernel(loops=1, collective=True, warm_big=24, warm_pre=2, dbg=False):
    nc = bacc.Bacc("TRN2", target_bir_lowering=False, debug=False,
                   num_devices=N_CORES)

    xin = nc.dram_tensor("xin", [B, C, NPC], F32, kind="ExternalInput")
    wconst = nc.dram_tensor("wconst", [C, WC_COLS], F32, kind="ExternalInput")
    auxidx = nc.dram_tensor("auxidx", [128, 32], I16, kind="ExternalInput")
    yout = nc.dram_tensor("yout", [B, C, NPC], F32, kind="ExternalOutput")
    dbgs = None
    if dbg:
        dbgs = {
            "d_ccsb": nc.dram_tensor("d_ccsb", [128, 64], F32, kind="ExternalOutput"),
            "d_ccout": nc.dram_tensor("d_ccout", [128, 64], F32, kind="ExternalOutput"),
            "d_ail": nc.dram_tensor("d_ail", [64, 128], F32, kind="ExternalOutput"),
            "d_tsb": nc.dram_tensor("d_tsb", [128, 256], F32, kind="ExternalOutput"),
            "d_lz0": nc.dram_tensor("d_lz0", [128, 64], F32, kind="ExternalOutput"),
            "d_lz1": nc.dram_tensor("d_lz1", [128, 64], F32, kind="ExternalOutput"),
            "d_z": nc.dram_tensor("d_z", [128, 1024], F32, kind="ExternalOutput"),
            "d_ybig": nc.dram_tensor("d_ybig", [128, 512], F32, kind="ExternalOutput"),
            "d_xs0": nc.dram_tensor("d_xs0", [128, 256], F32, kind="ExternalOutput"),
            "d_inv0": nc.dram_tensor("d_inv0", [128, 4], F32, kind="ExternalOutput"),
        }

    with tile.TileContext(nc) as tc:
        with (
            tc.tile_pool(name="consts", bufs=1) as consts,
            tc.tile_pool(name="data", bufs=2) as data,
            tc.tile_pool(name="small", bufs=2) as small,
            tc.tile_pool(name="pst", bufs=2, space="PSUM") as pst,
            tc.tile_pool(name="psa", bufs=1, space="PSUM") as psa,
            tc.tile_pool(name="psb", bufs=2, space="PSUM") as psb,
            tc.tile_pool(name="psw", bufs=2, space="PSUM") as psw,
            tc.tile_pool(name="dram", bufs=1, space="DRAM") as dram,
        ):
            pools = (data, small, consts, pst, psa, psb, psw, dram)
            tensors = {
                "xin": xin, "yout": yout, "wconst": wconst, "auxidx": auxidx,
                "collective": collective, "dbg": dbgs,
                "warm_big": warm_big, "warm_pre": warm_pre,
            }
            for _ in range(loops):
                _emit(nc, tc, pools, tensors)

    nc.compile()
    return nc


_NC_CACHE = {}


def _get_nc(loops=1, collective=True):
    key = (loops, collective)
    if key not in _NC_CACHE:
        _NC_CACHE[key] = build_kernel(loops=loops, collective=collective)
    return _NC_CACHE[key]


def _host_weights(g, w_qkv, w_out, b_out):
    Wp = w_qkv.astype(np.float64) * (8.0 * g.astype(np.float64))[None, :]
    WQ = Wp[0:HID] * SCALE
    WK = Wp[HID:2 * HID]
    WV = Wp[2 * HID:3 * HID]
    U = np.stack([
        w_out[:, 32 * h:32 * h + 32].astype(np.float64) @ WV[32 * h:32 * h + 32]
        for h in range(HEADS)
    ])  # [4, 64, 64], U_h = W_out_h @ WV_h
    V = np.stack([
        WK[32 * h:32 * h + 32].T @ WQ[32 * h:32 * h + 32]
        for h in range(HEADS)
    ])  # [4, 64, 64]
    wc = np.zeros((C, WC_COLS), dtype=np.float32)
    for h in range(HEADS):
        wc[:, h * C:(h + 1) * C] = U[h].T.astype(np.float32)          # ucatT
        wc[:, HEADS * C + h * C:HEADS * C + (h + 1) * C] = V[h].astype(np.float32)
    wc[:, 2 * HEADS * C] = np.asarray(b_out, np.float32)
    return np.ascontiguousarray(wc)


def _aux_idx():
    """SWDGE index tables, [128, 16] int16.

    cols 0:8  — identity map: listed index i -> row i        (gather/yout)
    cols 8:16 — export map: src partition i=(b*64+c) -> row (2c+b)
    The engine reads idx(i) from [i%16, i//16]; partitions 16.. are unread
    but must hold in-range values.
    """
    idx = np.zeros((128, 32), dtype=np.int16)
    for p in range(128):
        for s in range(8):
            i = s * 16 + (p % 16)
            idx[p, s] = i
            idx[p, 8 + s] = 2 * (i % 64) + (i // 64)
            idx[p, 16 + s] = i          # b0 output sub-rows
            idx[p, 24 + s] = 128 + i    # b1 output sub-rows
    return idx


def _in_maps(x, g, w_qkv, w_out, b_out):
    x = np.asarray(x, dtype=np.float32)
    b, c, h, w, d = x.shape
    n = h * w * d
    xf = np.ascontiguousarray(x.reshape(b, c, n))
    wc = _host_weights(
        np.asarray(g, np.float32), np.asarray(w_qkv, np.float32),
        np.asarray(w_out, np.float32), np.asarray(b_out, np.float32))
    aux = _aux_idx()
    maps = []
    for core in range(N_CORES):
        sl = np.ascontiguousarray(xf[:, :, core * NPC:(core + 1) * NPC])
        maps.append({"xin": sl, "wconst": wc, "auxidx": aux})
    return maps, (b, c, h, w, d, n)


def _gather_out(res, shape):
    b, c, h, w, d, n = shape
    out = np.empty((b, c, n), dtype=np.float32)
    for core in range(N_CORES):
        out[:, :, core * NPC:(core + 1) * NPC] = res.results[core]["yout"]
    return out.reshape(b, c, h, w, d)


def kernel(x, g, w_qkv, w_out, b_out, **_unused):
    maps, shape = _in_maps(x, g, w_qkv, w_out, b_out)
    nc = _get_nc()
    res = run_bass_kernel_spmd(nc, maps, core_ids=list(range(N_CORES)))
    return _gather_out(res, shape)


def run_variant(x, g, w_qkv, w_out, b_out, loops=1, collective=True, **kwargs):
    """Run a loop/collective variant; returns (out, BassKernelResults)."""
    maps, shape = _in_maps(x, g, w_qkv, w_out, b_out)
    nc = _get_nc(loops=loops, collective=collective)
    res = run_bass_kernel_spmd(nc, maps, core_ids=list(range(N_CORES)), **kwargs)
    return _gather_out(res, shape), res


# revision 40
# speedup vs baseline: 1.0169x; 1.0040x over previous
"""Trainium2 Bass kernel for nn_Attention_6073083756792.

The reference module is (faithfully) softmax-free: attn = sim = (q^T k), so the
whole attention block is linear in the normalized input.  Folding the RMSNorm
column scaling through the channel GEMMs collapses the entire module to

    y[:, j] = E_b @ xs[:, j] + b_out + x[:, j]      per batch b, where
    xs[:, j] = x[:, j] / ||x[:, j]||
    A_b  = sum_j xs_j xs_j^T                        (64 x 64 Gram matrix)
    E_b  = sum_h U_h @ A_b @ V_h                    (64 x 64)
    U_h  = W_out[:, h] @ WV_h          (host precomputed, weights only)
    V_h  = WK_h^T @ WQ_h               (host precomputed, weights only)

Device schedule per core (spatial columns sharded 8 ways, 512 cols/core/batch):
  phase 1: per-batch input DMAs on two queues (pipelined); PE transposes to
           j-major; ACT Square, DVE grouped reduce + reciprocal, ACT sqrt,
           one DVE mul -> xs (unit columns); Gram via stat = mov = xs;
           per-batch PSUM->SBUF copies on ACT/DVE.
  The Gram export and the final output use the SWDGE prepare/trigger path:
  descriptor generation runs on the gpsimd engine well ahead of time, so the
  triggered DMA costs only trigger + transfer + completion-sem instead of a
  full HWDGE issue cycle (~1.1us saved per hop).  cc_in / yout are pre-zeroed
  by early SWDGE copies so the scatter-ADD lands plain values.  (dma_gather
  is not functional under fake_nrt, so the import stays a plain DMA.)
  AllReduce (add) of the [128, 64] partial-Gram block (32 KB).
  phase 2: plain import DMA lands the reduced Gram in SBUF in (c, (b, k))
           layout via a strided AP; one stacked t-matmul; blkdiag(V_h,V_h)
           E-matmuls; y = [Ec ; I]^T @ [xs ; x] + b in float32r; output via a
           pre-prepared scatter triggered on the bias copies.
  The xs transpose-back path runs in slack time during the collective; junk
  matmuls keep PE out of the cold p-state across the collective window.
"""

import numpy as np

import concourse.bacc as bacc
import concourse.bass as bass
import concourse.mybir as mybir
import concourse.tile as tile
from concourse.bass_utils import run_bass_kernel_spmd
from concourse.masks import make_identity

F32 = mybir.dt.float32
F32R = mybir.dt.float32r
BF16 = mybir.dt.bfloat16
I16 = mybir.dt.int16
AF = mybir.ActivationFunctionType

N_CORES = 8
B = 2
C = 64          # channels (dim)
N = 4096        # spatial positions 16*16*16
NPC = N // N_CORES  # columns per core
NT = NPC // 128     # 128-column j-tiles per batch per core
HEADS = 4
DIM_HEAD = 32
HID = HEADS * DIM_HEAD
SCALE = DIM_HEAD ** -0.5
EPS = 1e-12     # torch F.normalize default (reference)

# packed const layout: [ucatT (256) | vflat (256) | bvec (1)]
WC_COLS = HEADS * C + HEADS * C + 1


def _emit(nc, tc, pools, tensors):
    data, small, consts, pst, psa, psb, psw, dram = pools
    xin, yout, wconst, auxidx = (
        tensors["xin"], tensors["yout"], tensors["wconst"], tensors["auxidx"])
    collective = tensors["collective"]
    warm_big, warm_pre = tensors["warm_big"], tensors["warm_pre"]

    yout_rows = yout[:, :, :].rearrange("b c n -> (b c) n")

    # ---- constants / setup (emitted first; engines idle before input lands)
    ident = consts.tile([128, 128], F32)
    make_identity(nc, ident[:, :])                      # Pool memset+select
    identr = consts.tile([128, 128], F32R)
    nc.vector.tensor_copy(identr[:, :], ident[:, :])    # DVE

    idx_sb = consts.tile([128, 32], I16)

    wc_sb = consts.tile([C, WC_COLS], F32R)
    b_sb = wc_sb[:, 2 * HEADS * C:2 * HEADS * C + 1].bitcast(F32)

    # [Ec ; I] stationaries: identity halves via partition-shifted DVE copies
    lzs = []
    for b in range(B):
        lz = consts.tile([2 * C, C], F32R, tag=f"lz{b}")
        nc.vector.tensor_copy(lz[C:2 * C, :], ident[0:C, 0:C])
        lzs.append(lz)

    zeros = consts.tile([128, NPC], F32)
    nc.vector.memset(zeros[:, :], 0.0)

    # preload the sqrt_and_others ACT table (covers Sqrt, Square, Identity,
    # Copy) while input DMAs are in flight
    warm = consts.tile([1, 1], F32)
    nc.vector.memset(warm[:, :], 0.0)
    nc.scalar.sqrt(warm[:, :], warm[:, :])

    # ---- input DMAs: one per batch on two queues so batch-0 compute starts
    # while batch-1 is still in flight
    z_sb = data.tile([2 * C, B * NPC], F32R, tag="z")
    nc.sync.dma_start(z_sb[C:2 * C, 0:NPC], xin[0, :, :].bitcast(F32R))
    nc.sync.dma_start(z_sb[C:2 * C, NPC:2 * NPC], xin[1, :, :].bitcast(F32R))
    nc.sync.dma_start(idx_sb[:, :], auxidx[:, :])

    # DRAM round-trip buffers for the collective ([128 rows, 64] layout:
    # row 2c+b holds A_b[c, :])
    cc_in = dram.tile([2 * C, C], F32, tag="cc_in")
    cc_out = dram.tile([2 * C, C], F32, tag="cc_out")

    # pre-zero cc_in / yout (plain SWDGE copies; the exports below ADD)
    nc.gpsimd.dma_start(cc_in[:, :], zeros[:, 0:C])
    nc.gpsimd.dma_start(yout_rows, zeros[:, :])
    nc.sync.dma_start(wc_sb[:, :], wconst[:, :].bitcast(F32R))

    # early PE warm-up: ramp the tensor engine before the input lands
    warm_ps = psw.tile([C, C], F32, tag="junk")
    for _ in range(warm_pre):
        nc.tensor.matmul(warm_ps[:, :], ident[0:C, 0:C], ident[0:C, 0:C],
                         start=True, stop=True)

    # ---- phase 1 ----
    xT_pss, sq_sbs, ss_sbs, inv_sbs, xs_sbs = [], [], [], [], []
    for b in range(B):
        xT_ps = pst.tile([128, NT * C], F32R, tag="xT")
        for i in range(NT):
            nc.tensor.transpose(
                xT_ps[:, i * C:(i + 1) * C],
                z_sb[C:2 * C, b * NPC + i * 128:b * NPC + (i + 1) * 128],
                identr[C:128, C:128],
            )
        xT_pss.append(xT_ps)

    for b in range(B):
        sq = data.tile([128, NT * C], F32, tag="sq")
        nc.scalar.activation(sq[:, :], xT_pss[b][:, :].bitcast(F32), AF.Square)
        sq_sbs.append(sq)

    for b in range(B):
        ss = small.tile([128, NT], F32, tag="ss")
        nc.vector.tensor_reduce(
            ss[:, :],
            sq_sbs[b][:, :].rearrange("p (g k) -> p g k", g=NT),
            mybir.AxisListType.X,
            mybir.AluOpType.add,
        )
        ss_sbs.append(ss)

    for b in range(B):
        sroot = small.tile([128, NT], F32, tag="sroot")
        nc.scalar.sqrt(sroot[:, :], ss_sbs[b][:, :])    # ||x_j|| on ACT
        inv_sbs.append(sroot)

    invr_sbs = []
    for b in range(B):
        invr = small.tile([128, NT], F32, tag="invr")
        nc.vector.reciprocal(invr[:, :], inv_sbs[b][:, :])  # 1/||x_j|| on DVE
        invr_sbs.append(invr)

    for b in range(B):
        xs = data.tile([128, NT * C], F32, tag="xs")
        nc.vector.tensor_mul(
            xs[:, :].rearrange("p (g k) -> p g k", g=NT),
            xT_pss[b][:, :].bitcast(F32).rearrange("p (g k) -> p g k", g=NT),
            invr_sbs[b][:, :].unsqueeze(2).broadcast_to((128, NT, C)),
        )
        xs_sbs.append(xs)

    a_pss = []
    for b in range(B):
        a_ps = psa.tile([C, C], F32, tag=f"A{b}")
        for i in range(NT):
            nc.tensor.matmul(
                a_ps[:, :],
                xs_sbs[b][:, i * C:(i + 1) * C],
                xs_sbs[b][:, i * C:(i + 1) * C],
                start=(i == 0), stop=(i == NT - 1),
            )
        a_pss.append(a_ps)

    # partial-Gram staging: batch halves on parallel engines (partition-
    # stacked [2C, C] so one export covers both)
    cc_sb = small.tile([2 * C, C], F32, tag="cc_sb")
    nc.scalar.copy(cc_sb[0:C, :], a_pss[0][:, :])
    nc.vector.tensor_copy(cc_sb[C:2 * C, :], a_pss[1][:, :])

    # export: prepared scatter-add (desc-gen ran long ago on gpsimd), fired
    # the moment both Gram halves are staged.  src partition (b*64+c) lands
    # on cc_in row (2c+b).
    sem_exp = nc.alloc_semaphore("exp_dma")
    nc.gpsimd.dma_scatter_add(
        cc_in[:, :], cc_sb[:, :].rearrange("p (n e) -> p n e", n=1),
        idx_sb[:, 8:16], 128, 128, C,
        prepare_only=True, sem=sem_exp)
    nc.gpsimd.trigger_dma(count=None)

    if collective:
        nc.gpsimd.collective_compute(
            "AllReduce",
            mybir.AluOpType.add,
            replica_groups=[list(range(N_CORES))],
            ins=[cc_in.opt()],
            outs=[cc_out.opt()],
        )
    else:
        # timing-model variant: stand-in DMA instead of the collective
        nc.sync.dma_start(cc_out[:, :], cc_in[:, :])

    # import: plain DMA; row (2c+b) lands at a_il[c, b, :], giving the
    # batch-stacked stationary [c, (b k)] for the t-matmul in SBUF.
    a_il = small.tile([C, B * C], F32R, tag="a_il")
    nc.sync.dma_start(
        a_il[:, :].rearrange("p (n e) -> p n e", n=B),
        cc_out[:, :].bitcast(F32R).rearrange("(c two) k -> c two k", two=2),
    )

    # V stationaries (bf16), replicated on both partition halves so the
    # per-batch E matmuls use matched stat/mov partition offsets
    vrep = consts.tile([2 * C, HEADS * C], BF16)
    wcv = wc_sb[:, HEADS * C:2 * HEADS * C].bitcast(F32)
    nc.scalar.copy(vrep[0:C, :], wcv)
    nc.scalar.copy(vrep[C:2 * C, :], wcv)
    b_sb2 = consts.tile([2 * C, 1], F32)
    nc.gpsimd.tensor_copy(b_sb2[0:C, :], b_sb)
    nc.gpsimd.tensor_copy(b_sb2[C:2 * C, :], b_sb)

    # slack path: xs transposed back to channel-major into z rows 0:64 (only
    # needed by the post-collective apply)
    for b in range(B):
        tb_ps = psb.tile([C, NPC], F32, tag="tb")
        for i in range(NT):
            nc.tensor.transpose(
                tb_ps[:, i * 128:(i + 1) * 128],
                xs_sbs[b][:, i * C:(i + 1) * C],
                ident[:, :],
            )
        if b == 0:
            nc.scalar.copy(z_sb[0:C, 0:NPC], tb_ps[:, :])
        else:
            nc.vector.tensor_copy(z_sb[0:C, NPC:2 * NPC], tb_ps[:, :])

    # PE keep-warm filler across the collective window (the cost model drops
    # PE to a cold p-state after ~3us idle).  Gated on cc_sb so they cannot
    # preempt phase-1 PE work; WAW-serialized on one scratch ring.
    for _ in range(warm_big):
        junk = psw.tile([C, C], F32, tag="junk")
        nc.tensor.matmul(junk[:, :], cc_sb[0:C, 0:C], cc_sb[0:C, 0:C],
                         start=True, stop=True)

    # ---- phase 2: E chain + apply ----
    t_full = pst.tile([128, HEADS * C], F32R, tag="xT")
    t_ps = t_full[:, :].bitcast(F32)
    nc.tensor.matmul(t_ps, a_il[:, :], wc_sb[:, 0:HEADS * C])
    t_sb = small.tile([128, HEADS * C], BF16, tag="t_sb")
    nc.vector.tensor_copy(t_sb[:, :], t_ps[:, :])

    e0_ps = psa.tile([C, C], F32, tag="A0")
    e1_ps = psa.tile([C, C], F32, tag="A1")
    for h in range(HEADS):
        nc.tensor.matmul(
            e0_ps[:, :], vrep[0:C, h * C:(h + 1) * C],
            t_sb[0:C, h * C:(h + 1) * C],
            start=(h == 0), stop=(h == HEADS - 1),
        )
        nc.tensor.matmul(
            e1_ps[:, :], vrep[C:2 * C, h * C:(h + 1) * C],
            t_sb[C:2 * C, h * C:(h + 1) * C],
            start=(h == 0), stop=(h == HEADS - 1),
        )
    nc.scalar.copy(lzs[1][0:C, :], e1_ps[:, :])
    nc.vector.tensor_copy(lzs[0][0:C, :], e0_ps[:, :])

    y_pss = [None, None]
    for b in (1, 0):
        y_ps = psb.tile([C, NPC], F32, tag="tb")
        nc.tensor.matmul(y_ps[:, :], lzs[b][:, :],
                         z_sb[:, b * NPC:(b + 1) * NPC])
        y_pss[b] = y_ps

    # bias + PSUM->SBUF staging on parallel engines into one [128, 512] tile
    ybig = data.tile([2 * C, NPC], F32, tag="ybig")
    nc.scalar.activation(ybig[0:C, :], y_pss[0][:, :], AF.Identity,
                         bias=b_sb2[0:C, :], scale=1.0)
    nc.vector.tensor_scalar_add(ybig[C:2 * C, :], y_pss[1][:, :],
                                b_sb2[0:C, :])

    # output: prepared scatter-add into the pre-zeroed yout, triggered on the
    # bias copies.  (Per-batch scatters were tried and are slower: Tile WAW-
    # serializes same-tensor scatters through the full DMA completion.)
    sem_y = nc.alloc_semaphore("y_dma")
    nc.gpsimd.dma_scatter_add(
        yout_rows, ybig[:, :].rearrange("p (n e) -> p n e", n=1),
        idx_sb[:, 0:8], 128, 128, NPC,
        prepare_only=True, sem=sem_y)
    nc.gpsimd.trigger_dma(count=None)

    if tensors.get("dbg"):
        dbgs = tensors["dbg"]
        nc.sync.dma_start(dbgs["d_ccsb"][:, :], cc_sb[:, :])
        nc.sync.dma_start(dbgs["d_ccout"][:, :], cc_out[:, :])
        nc.sync.dma_start(dbgs["d_ail"][:, :], a_il[:, :].bitcast(F32))
        nc.sync.dma_start(dbgs["d_tsb"][:, :], t_sb[:, :])
        nc.sync.dma_start(dbgs["d_lz0"][:, :], lzs[0][:, :].bitcast(F32))
        nc.sync.dma_start(dbgs["d_lz1"][:, :], lzs[1][:, :].bitcast(F32))
        nc.sync.dma_start(dbgs["d_z"][:, :], z_sb[:, :].bitcast(F32))
        nc.sync.dma_start(dbgs["d_ybig"][:, :], ybig[:, :])
        nc.sync.dma_start(dbgs["d_xs0"][:, :], xs_sbs[0][:, :])
        nc.sync.dma_start(dbgs["d_inv0"][:, :], invr_sbs[0][:, :])


def build_kernel(loops=1, collective=True, warm_big=24, warm_pre=2, dbg=False):
    nc = bacc.Bacc("TRN2", target_bir_lowering=False, debug=False,
                   num_devices=N_CORES)

    xin = nc.dram_tensor("xin", [B, C, NPC], F32, kind="ExternalInput")
    wconst = nc.dram_tensor("wconst", [C, WC_COLS], F32, kind="ExternalInput")
    auxidx = nc.dram_tensor("auxidx", [128, 32], I16, kind="ExternalInput")
    yout = nc.dram_tensor("yout", [B, C, NPC], F32, kind="ExternalOutput")
    dbgs = None
    if dbg:
        dbgs = {
            "d_ccsb": nc.dram_tensor("d_ccsb", [128, 64], F32, kind="ExternalOutput"),
            "d_ccout": nc.dram_tensor("d_ccout", [128, 64], F32, kind="ExternalOutput"),
            "d_ail": nc.dram_tensor("d_ail", [64, 128], F32, kind="ExternalOutput"),
            "d_tsb": nc.dram_tensor("d_tsb", [128, 256], F32, kind="ExternalOutput"),
            "d_lz0": nc.dram_tensor("d_lz0", [128, 64], F32, kind="ExternalOutput"),
            "d_lz1": nc.dram_tensor("d_lz1", [128, 64], F32, kind="ExternalOutput"),
            "d_z": nc.dram_tensor("d_z", [128, 1024], F32, kind="ExternalOutput"),
            "d_ybig": nc.dram_tensor("d_ybig", [128, 512], F32, kind="ExternalOutput"),
            "d_xs0": nc.dram_tensor("d_xs0", [128, 256], F32, kind="ExternalOutput"),
            "d_inv0": nc.dram_tensor("d_inv0", [128, 4], F32, kind="ExternalOutput"),
        }

    with tile.TileContext(nc) as tc:
        with (
            tc.tile_pool(name="consts", bufs=1) as consts,
            tc.tile_pool(name="data", bufs=2) as data,
            tc.tile_pool(name="small", bufs=2) as small,
            tc.tile_pool(name="pst", bufs=2, space="PSUM") as pst,
            tc.tile_pool(name="psa", bufs=1, space="PSUM") as psa,
            tc.tile_pool(name="psb", bufs=2, space="PSUM") as psb,
            tc.tile_pool(name="psw", bufs=2, space="PSUM") as psw,
            tc.tile_pool(name="dram", bufs=1, space="DRAM") as dram,
        ):
            pools = (data, small, consts, pst, psa, psb, psw, dram)
            tensors = {
                "xin": xin, "yout": yout, "wconst": wconst, "auxidx": auxidx,
                "collective": collective, "dbg": dbgs,
                "warm_big": warm_big, "warm_pre": warm_pre,
            }
            for _ in range(loops):
                _emit(nc, tc, pools, tensors)

    nc.compile()
    return nc


_NC_CACHE = {}


def _get_nc(loops=1, collective=True):
    key = (loops, collective)
    if key not in _NC_CACHE:
        _NC_CACHE[key] = build_kernel(loops=loops, collective=collective)
    return _NC_CACHE[key]


def _host_weights(g, w_qkv, w_out, b_out):
    Wp = w_qkv.astype(np.float64) * (8.0 * g.astype(np.float64))[None, :]
    WQ = Wp[0:HID] * SCALE
    WK = Wp[HID:2 * HID]
    WV = Wp[2 * HID:3 * HID]
    U = np.stack([
        w_out[:, 32 * h:32 * h + 32].astype(np.float64) @ WV[32 * h:32 * h + 32]
        for h in range(HEADS)
    ])  # [4, 64, 64], U_h = W_out_h @ WV_h
    V = np.stack([
        WK[32 * h:32 * h + 32].T @ WQ[32 * h:32 * h + 32]
        for h in range(HEADS)
    ])  # [4, 64, 64]
    wc = np.zeros((C, WC_COLS), dtype=np.float32)
    for h in range(HEADS):
        wc[:, h * C:(h + 1) * C] = U[h].T.astype(np.float32)          # ucatT
        wc[:, HEADS * C + h * C:HEADS * C + (h + 1) * C] = V[h].astype(np.float32)
    wc[:, 2 * HEADS * C] = np.asarray(b_out, np.float32)
    return np.ascontiguousarray(wc)


def _aux_idx():
    """SWDGE index tables, [128, 16] int16.

    cols 0:8  — identity map: listed index i -> row i        (gather/yout)
    cols 8:16 — export map: src partition i=(b*64+c) -> row (2c+b)
    The engine reads idx(i) from [i%16, i//16]; partitions 16.. are unread
    but must hold in-range values.
    """
    idx = np.zeros((128, 32), dtype=np.int16)
    for p in range(128):
        for s in range(8):
            i = s * 16 + (p % 16)
            idx[p, s] = i
            idx[p, 8 + s] = 2 * (i % 64) + (i // 64)
            idx[p, 16 + s] = i          # b0 output sub-rows
            idx[p, 24 + s] = 128 + i    # b1 output sub-rows
    return idx


def _in_maps(x, g, w_qkv, w_out, b_out):
    x = np.asarray(x, dtype=np.float32)
    b, c, h, w, d = x.shape
    n = h * w * d
    xf = np.ascontiguousarray(x.reshape(b, c, n))
    wc = _host_weights(
        np.asarray(g, np.float32), np.asarray(w_qkv, np.float32),
        np.asarray(w_out, np.float32), np.asarray(b_out, np.float32))
    aux = _aux_idx()
    maps = []
    for core in range(N_CORES):
        sl = np.ascontiguousarray(xf[:, :, core * NPC:(core + 1) * NPC])
        maps.append({"xin": sl, "wconst": wc, "auxidx": aux})
    return maps, (b, c, h, w, d, n)


def _gather_out(res, shape):
    b, c, h, w, d, n = shape
    out = np.empty((b, c, n), dtype=np.float32)
    for core in range(N_CORES):
        out[:, :, core * NPC:(core + 1) * NPC] = res.results[core]["yout"]
    return out.reshape(b, c, h, w, d)


def kernel(x, g, w_qkv, w_out, b_out, **_unused):
    maps, shape = _in_maps(x, g, w_qkv, w_out, b_out)
    nc = _get_nc()
    res = run_bass_kernel_spmd(nc, maps, core_ids=list(range(N_CORES)))
    return _gather_out(res, shape)


def run_variant(x, g, w_qkv, w_out, b_out, loops=1, collective=True, **kwargs):
    """Run a loop/collective variant; returns (out, BassKernelResults)."""
    maps, shape = _in_maps(x, g, w_qkv, w_out, b_out)
    nc = _get_nc(loops=loops, collective=collective)
    res = run_bass_kernel_spmd(nc, maps, core_ids=list(range(N_CORES)), **kwargs)
    return _gather_out(res, shape), res
